# revision 1
# baseline (speedup 1.0000x reference)
"""GQA FlashAttention (RMSNorm QK + RoPE, causal) on 8 TRN2 NeuronCores.

Sharding: tensor-parallel over heads for QKV projection + attention
(core c owns q-heads 4c..4c+3 and kv-head c — the GQA group is fully
local, so attention needs no collective). A single AllToAll re-shards
the attention output from head-parallel to seq-row-parallel, after
which each core computes its 256 output rows against the full Wo
(no all-reduce). Softmax uses the unnormalized-exp trick: denominators
come free from a ones-column appended to V, and the division is applied
to the small attention output after the PV matmul.

All matmuls run in float32r (fp32 storage, ~4x fp32 PE rate; measured
same precision as the fp32 PE path). Everything is computed in the
transposed layout (head_dim on partitions) so the scores output IS the
P^T operand the PV matmul needs — zero transposes in the attention
inner loop.
"""

import sys

sys.path.insert(0, "/opt/trn_rl_repo")

import numpy as np
import concourse.bass as bass  # noqa: F401  (engine types referenced via nc)
import concourse.tile as tile
from concourse import mybir, bacc
from concourse.bass_utils import run_bass_kernel_spmd
from concourse.masks import make_identity

N_CORES = 8
D_IN = 2048
SEQ = 2048
N_HEADS = 32
N_KV = 8
HD = 64
HPC = N_HEADS // N_CORES  # 4 q heads per core
EPS = 1e-6
NEG = -1.0e9

F32 = mybir.dt.float32
F32R = mybir.dt.float32r

KT_TILES = D_IN // 128  # 16 contraction tiles for projections
QB = 512  # q block (matmul moving dim)
NQB = SEQ // QB  # 4
NKT = SEQ // 128  # 16 kv tiles
ROWS_PER_CORE = SEQ // N_CORES  # 256


def _build():
    nc = bacc.Bacc(num_devices=N_CORES)

    xT = nc.dram_tensor("xT", [D_IN, SEQ], F32R, kind="ExternalInput")
    wq = nc.dram_tensor("wq", [D_IN, HPC * HD], F32R, kind="ExternalInput")
    wkv = nc.dram_tensor("wkv", [D_IN, 2 * HD], F32R, kind="ExternalInput")
    wo = nc.dram_tensor("wo", [D_IN, D_IN], F32R, kind="ExternalInput")
    cosT2 = nc.dram_tensor("cosT2", [128, SEQ], F32, kind="ExternalInput")
    sinT2 = nc.dram_tensor("sinT2", [128, SEQ], F32, kind="ExternalInput")
    qw2 = nc.dram_tensor("qw2", [128, 1], F32, kind="ExternalInput")
    kw = nc.dram_tensor("kw", [64, 1], F32, kind="ExternalInput")
    tri = nc.dram_tensor("tri", [128, 128], F32, kind="ExternalInput")
    sel = nc.dram_tensor("sel", [4 * N_CORES, 2 * N_CORES, 128], F32R, kind="ExternalInput")
    onesblk_in = nc.dram_tensor("onesblk", [128, 128], F32R, kind="ExternalInput")
    onescol_in = nc.dram_tensor("onescol", [128, 1], F32R, kind="ExternalInput")

    out = nc.dram_tensor("out", [ROWS_PER_CORE, D_IN], F32, kind="ExternalOutput")

    with tile.TileContext(nc) as tc:
        with (
            tc.tile_pool(name="persist", bufs=1) as pers,
            tc.tile_pool(name="dram", bufs=1, space="DRAM") as dram,
        ):
            # ---- persistent SBUF ----
            wq_sb = pers.tile([128, KT_TILES, HPC * HD], F32R)  # 2 MB
            nc.sync.dma_start(
                wq_sb[:], wq.rearrange("(ko p) m -> p ko m", p=128)
            )
            wkv_sb = pers.tile([128, KT_TILES, 2 * HD], F32R)  # 1 MB
            nc.sync.dma_start(
                wkv_sb[:], wkv.rearrange("(ko p) m -> p ko m", p=128)
            )
            cos_sb = pers.tile([128, SEQ], F32)
            sin_sb = pers.tile([128, SEQ], F32)
            nc.sync.dma_start(cos_sb[:], cosT2[:])
            nc.sync.dma_start(sin_sb[:], sinT2[:])
            qw_sb = pers.tile([128, 1], F32)
            kw_sb = pers.tile([64, 1], F32)
            nc.sync.dma_start(qw_sb[:], qw2[:])
            nc.sync.dma_start(kw_sb[:], kw[:])
            tri_sb = pers.tile([128, 128], F32)
            nc.sync.dma_start(tri_sb[:], tri[:])
            eps_sb = pers.tile([128, 1], F32)
            nc.vector.memset(eps_sb[:], EPS)
            sel_sb = pers.tile([4 * N_CORES, 2 * N_CORES, 128], F32R)
            nc.sync.dma_start(sel_sb[:], sel[:])

            ident = pers.tile([128, 128], F32)
            make_identity(nc, ident[:])

            # block-diagonal ones (two 64x64 blocks) for per-head sumsq+broadcast
            onesblk = pers.tile([128, 128], F32R)
            nc.sync.dma_start(onesblk[:], onesblk_in[:])

            # QT per head at base partition 0: [64, 4 heads, SEQ]
            qt_sb = pers.tile([64, HPC, SEQ], F32R)  # 2 MB
            kt_sb = pers.tile([64, SEQ], F32R)  # 0.5 MB
            vaug_sb = pers.tile([128, NKT, HD + 1], F32R)  # 0.53 MB
            for _t in range(NKT):
                nc.sync.dma_start(vaug_sb[:, _t, HD : HD + 1], onescol_in[:])

            # DRAM scratch for the AllToAll
            a2a_in = dram.tile([N_CORES, HPC * HD + HPC, ROWS_PER_CORE], F32)
            a2a_out = dram.tile([N_CORES, HPC * HD + HPC, ROWS_PER_CORE], F32)

            # ================= Phase 1: projections + norm + rope =============
            with (
                tc.tile_pool(name="xt", bufs=4) as xp,
                tc.tile_pool(name="p1ps", bufs=2, space="PSUM") as psA,
                tc.tile_pool(name="p1sb", bufs=3) as t1,
            ):
                for j in range(NQB):
                    sl = slice(QB * j, QB * j + QB)
                    acc = [
                        psA.tile([128, QB], F32, tag="acc0", name=f"acc0_{j}"),
                        psA.tile([128, QB], F32, tag="acc1", name=f"acc1_{j}"),
                        psA.tile([128, QB], F32, tag="acc2", name=f"acc2_{j}"),
                    ]
                    for k in range(KT_TILES):
                        xt = xp.tile([128, QB], F32R, tag="xt")
                        nc.sync.dma_start(
                            xt[:], xT[128 * k : 128 * k + 128, sl]
                        )
                        st = k == 0
                        sp = k == KT_TILES - 1
                        nc.tensor.matmul(
                            acc[0][:], wq_sb[:, k, 0:128], xt[:], start=st, stop=sp
                        )
                        nc.tensor.matmul(
                            acc[1][:], wq_sb[:, k, 128:256], xt[:], start=st, stop=sp
                        )
                        nc.tensor.matmul(
                            acc[2][:], wkv_sb[:, k, :], xt[:], start=st, stop=sp
                        )

                    for idx in range(3):
                        raw = acc[idx]
                        is_kv = idx == 2
                        rows = slice(0, 64) if is_kv else slice(0, 128)
                        # sumsq broadcast per head (block-diag ones matmul)
                        sq = t1.tile([128, QB], F32R, tag="sq")
                        nc.scalar.square(sq[:], raw[:])
                        psn = psA.tile([128, QB], F32, tag="norm", bufs=1)
                        nc.tensor.matmul(
                            psn[:], onesblk[:], sq[:], start=True, stop=True
                        )
                        rcp = t1.tile([128, QB], F32, tag="rcp")
                        nc.scalar.activation(
                            out=rcp[rows, :],
                            in_=psn[rows, :],
                            func=mybir.ActivationFunctionType.Sqrt,
                            bias=eps_sb[rows, :],
                            scale=1.0 / HD,
                        )
                        nc.vector.reciprocal(rcp[rows, :], rcp[rows, :])
                        # normalized = raw * rcp * norm_w
                        tn = t1.tile([128, QB], F32, tag="tn")
                        nc.vector.tensor_mul(tn[rows, :], raw[rows, :], rcp[rows, :])
                        if is_kv:
                            nc.vector.tensor_scalar_mul(
                                tn[0:64, :], tn[0:64, :], kw_sb[:]
                            )
                        else:
                            nc.vector.tensor_scalar_mul(tn[:], tn[:], qw_sb[:])
                        # rope: rot = [-t[32:64], t[0:32]] per 64-row head
                        rot = t1.tile([128, QB], F32, tag="rot")
                        nheads_here = 1 if is_kv else 2
                        for b in range(nheads_here):
                            o = 64 * b
                            nc.vector.tensor_scalar_mul(
                                rot[o : o + 32, :], tn[o + 32 : o + 64, :], -1.0
                            )
                            nc.vector.tensor_copy(
                                rot[o + 32 : o + 64, :], tn[o : o + 32, :]
                            )
                        if is_kv:
                            dst = kt_sb[:, sl]
                            nc.vector.tensor_mul(dst, tn[0:64, :], cos_sb[0:64, sl])
                            nc.vector.tensor_mul(
                                rot[0:64, :], rot[0:64, :], sin_sb[0:64, sl]
                            )
                            nc.vector.tensor_add(dst, dst, rot[0:64, :])
                            # V rows: evict + transpose to natural layout
                            vt = t1.tile([64, QB], F32, tag="vt")
                            nc.scalar.copy(vt[:], raw[64:128, :])
                            for ttl in range(QB // 128):
                                tg = (QB // 128) * j + ttl
                                psv = psA.tile([128, 64], F32, tag="vtr", bufs=1)
                                nc.tensor.transpose(
                                    psv[:],
                                    vt[:, 128 * ttl : 128 * ttl + 128],
                                    ident[0:64, 0:64],
                                )
                                nc.scalar.copy(vaug_sb[:, tg, 0:HD], psv[:])
                        else:
                            tmpc = t1.tile([128, QB], F32, tag="tmpc")
                            nc.vector.tensor_mul(tmpc[:], tn[:], cos_sb[:, sl])
                            nc.vector.tensor_mul(rot[:], rot[:], sin_sb[:, sl])
                            for b in range(2):
                                nc.vector.tensor_add(
                                    qt_sb[:, 2 * idx + b, sl],
                                    tmpc[64 * b : 64 * b + 64, :],
                                    rot[64 * b : 64 * b + 64, :],
                                )

            # ================= Phase 3: attention =============================
            with (
                tc.tile_pool(name="p3ps", bufs=3, space="PSUM") as psB,
                tc.tile_pool(name="p3pv", bufs=2, space="PSUM") as psPV,
                tc.tile_pool(name="p3sb", bufs=3) as t3,
            ):
                for h in range(HPC):
                    for j in range(NQB):
                        ntile = (QB // 128) * (j + 1)
                        pv = psPV.tile([128, QB], F32, tag="pv")
                        for t in range(ntile):
                            diag_m = t - (QB // 128) * j
                            ks = slice(128 * t, 128 * t + 128)
                            if diag_m < 0:
                                qs = slice(QB * j, QB * j + QB)
                                n0 = 0
                            else:
                                n0 = 128 * diag_m
                                qs = slice(QB * j + n0, QB * j + QB)
                            ps_s = psB.tile([128, QB], F32, tag="sc")
                            nc.tensor.matmul(
                                ps_s[:, 0 : QB - n0],
                                kt_sb[:, ks],
                                qt_sb[:, h, qs],
                                start=True,
                                stop=True,
                            )
                            if diag_m >= 0:
                                nc.vector.tensor_add(
                                    ps_s[:, 0:128], ps_s[:, 0:128], tri_sb[:]
                                )
                            pt = t3.tile([128, QB], F32R, tag="pt")
                            nc.scalar.activation(
                                out=pt[:, 0 : QB - n0],
                                in_=ps_s[:, 0 : QB - n0],
                                func=mybir.ActivationFunctionType.Exp,
                                scale=0.125,
                            )
                            nc.tensor.matmul(
                                pv[0:65, n0:QB],
                                vaug_sb[:, t, :],
                                pt[:, 0 : QB - n0],
                                start=(t == 0),
                                stop=(t == ntile - 1),
                            )
                        att = t3.tile([65, QB], F32, tag="att")
                        nc.scalar.copy(att[:], pv[0:65, :])
                        for s in range(QB // ROWS_PER_CORE):
                            shard = (QB // ROWS_PER_CORE) * j + s
                            cs = slice(ROWS_PER_CORE * s, ROWS_PER_CORE * (s + 1))
                            nc.sync.dma_start(
                                a2a_in[shard, 64 * h : 64 * h + 64, :],
                                att[0:64, cs],
                            )
                            nc.sync.dma_start(
                                a2a_in[shard, HPC * 64 + h, :], att[64:65, cs]
                            )

            # ================= Phase 4: AllToAll ==============================
            nc.gpsimd.collective_compute(
                "AllToAll",
                mybir.AluOpType.bypass,
                replica_groups=[list(range(N_CORES))],
                ins=[a2a_in[:].opt()],
                outs=[a2a_out[:].opt()],
            )

            # ================= Phase 5: out projection ========================
            with (
                tc.tile_pool(name="p5ps", bufs=2, space="PSUM") as psC,
                tc.tile_pool(name="p5bc", bufs=2, space="PSUM") as psD,
                tc.tile_pool(name="wo", bufs=8) as wop,
                tc.tile_pool(name="p5sb", bufs=4) as t5,
                tc.tile_pool(name="an", bufs=1) as anp,
            ):
                R = ROWS_PER_CORE
                dsb_raw = t5.tile([4 * N_CORES, R], F32, tag="denraw")
                for g in range(N_CORES):
                    nc.sync.dma_start(
                        dsb_raw[4 * g : 4 * g + 4, :],
                        a2a_out[g, HPC * 64 : HPC * 64 + 4, :],
                    )
                nc.vector.reciprocal(dsb_raw[:], dsb_raw[:])
                dsb = t5.tile([4 * N_CORES, R], F32R, tag="den")
                nc.vector.tensor_copy(dsb[:], dsb_raw[:])

                an_sb = anp.tile([128, 2 * N_CORES, R], F32R)  # normalized attnT
                for g in range(N_CORES):
                    for half in range(2):
                        a_raw = t5.tile([128, R], F32, tag="araw")
                        nc.sync.dma_start(
                            a_raw[:], a2a_out[g, 128 * half : 128 * half + 128, :]
                        )
                        bc = psD.tile([128, R], F32, tag="bc")
                        nc.tensor.matmul(
                            bc[:],
                            sel_sb[:, 2 * g + half, :],
                            dsb[:],
                            start=True,
                            stop=True,
                        )
                        nc.vector.tensor_mul(
                            an_sb[:, 2 * g + half, :], a_raw[:], bc[:]
                        )

                NB_OUT = D_IN // 512  # 4
                for nb in range(NB_OUT):
                    osl = slice(512 * nb, 512 * nb + 512)
                    po = [
                        psC.tile([128, 512], F32, tag="o0", name=f"o0_{nb}"),
                        psC.tile([128, 512], F32, tag="o1", name=f"o1_{nb}"),
                    ]
                    for gh in range(2 * N_CORES):
                        wt = wop.tile([128, 512], F32R, tag="wo")
                        nc.sync.dma_start(
                            wt[:], wo[128 * gh : 128 * gh + 128, osl]
                        )
                        for qt in range(2):
                            nc.tensor.matmul(
                                po[qt][:],
                                an_sb[:, gh, 128 * qt : 128 * qt + 128],
                                wt[:],
                                start=(gh == 0),
                                stop=(gh == 2 * N_CORES - 1),
                            )
                    for qt in range(2):
                        osb = t5.tile([128, 512], F32, tag="osb")
                        nc.scalar.copy(osb[:], po[qt][:])
                        nc.sync.dma_start(
                            out[128 * qt : 128 * qt + 128, osl], osb[:]
                        )

    nc.compile()
    return nc


_NC_CACHE = None


def _get_nc():
    global _NC_CACHE
    if _NC_CACHE is None:
        _NC_CACHE = _build()
    return _NC_CACHE


def _make_in_maps(x, cos, sin, wq, wk, wv, wo, q_norm_w, k_norm_w):
    x = np.asarray(x, dtype=np.float32)
    cos = np.asarray(cos, dtype=np.float32)
    sin = np.asarray(sin, dtype=np.float32)
    wq = np.asarray(wq, dtype=np.float32)
    wk = np.asarray(wk, dtype=np.float32)
    wv = np.asarray(wv, dtype=np.float32)
    wo = np.asarray(wo, dtype=np.float32)
    q_norm_w = np.asarray(q_norm_w, dtype=np.float32)
    k_norm_w = np.asarray(k_norm_w, dtype=np.float32)

    xT = np.ascontiguousarray(x[0].T)  # [D_IN, SEQ]
    cosT2 = np.ascontiguousarray(np.vstack([cos.T, cos.T]))  # [128, SEQ]
    sinT2 = np.ascontiguousarray(np.vstack([sin.T, sin.T]))
    qw2 = np.ascontiguousarray(np.concatenate([q_norm_w, q_norm_w])[:, None])
    kw1 = np.ascontiguousarray(k_norm_w[:, None])
    ii, jj = np.meshgrid(np.arange(128), np.arange(128), indexing="ij")
    tri = np.where(ii <= jj, 0.0, NEG).astype(np.float32)  # keep kv<=q
    onesblk = np.zeros((128, 128), np.float32)
    onesblk[0:64, 0:64] = 1.0
    onesblk[64:128, 64:128] = 1.0
    onescol = np.ones((128, 1), np.float32)
    sel = np.zeros((4 * N_CORES, 2 * N_CORES, 128), np.float32)
    for g in range(N_CORES):
        for half in range(2):
            for m in range(128):
                sel[4 * g + 2 * half + m // 64, 2 * g + half, m] = 1.0

    in_maps = []
    for c in range(N_CORES):
        wq_c = np.ascontiguousarray(wq[:, 256 * c : 256 * c + 256])
        wkv_c = np.ascontiguousarray(
            np.concatenate(
                [wk[:, 64 * c : 64 * c + 64], wv[:, 64 * c : 64 * c + 64]], axis=1
            )
        )
        in_maps.append(
            {
                "xT": xT,
                "wq": wq_c,
                "wkv": wkv_c,
                "wo": wo,
                "cosT2": cosT2,
                "sinT2": sinT2,
                "qw2": qw2,
                "kw": kw1,
                "tri": tri,
                "sel": sel,
                "onesblk": onesblk,
                "onescol": onescol,
            }
        )
    return in_maps


def kernel(x, cos, sin, wq, wk, wv, wo, q_norm_w, k_norm_w):
    in_maps = _make_in_maps(x, cos, sin, wq, wk, wv, wo, q_norm_w, k_norm_w)
    nc = _get_nc()
    res = run_bass_kernel_spmd(nc, in_maps, core_ids=list(range(N_CORES)))
    rows = [res.results[c]["out"] for c in range(N_CORES)]
    full = np.concatenate(rows, axis=0)  # [SEQ, D_IN]
    return full.reshape(1, SEQ, D_IN).astype(np.float32)



# revision 4
# speedup vs baseline: 1.1938x; 1.1938x over previous
"""GQA FlashAttention (RMSNorm QK + RoPE, causal) on 8 TRN2 NeuronCores.

Sharding: tensor-parallel over heads (core c owns q-heads 4c..4c+3 and
kv-head c; the GQA group is fully local). A single AllToAll re-shards the
attention output from head-parallel to row-parallel; each core then
computes its 256 output rows against the full Wo.

v2 vs baseline:
- bf16 for x/wq/wkv/wo and all on-chip matmul operands (1 cyc/row on PE,
  FWL weight loads, half the DMA bytes). PSUM accumulation stays fp32.
- Fused j-outer loop: projections for block j+1 interleave with attention
  for block j, keeping the PE dense so the HAM clock stays at 2.4 GHz
  (the baseline ran throttled at 1.2 GHz for 310us straight).
- RMSNorm rsqrt computed as exp(-0.5*ln(var+eps)) so the ACT engine needs
  only the natural_log_exp table set - zero table switches kernel-wide.
- Softmax exp batched over [128, 1024] PSUM spans (2 kv tiles per
  ACTIVATE) to amortize the ~185ns per-instruction ACT bubble.
- PSUM accumulators evicted to SBUF right after the projection chain so
  the 8 banks split 3 (proj acc) + 4 (scores, 2x double-buffered) + 1 (PV).
- Full Wo preloaded to SBUF (bf16, 8 MB) during compute; out-projection
  never waits on DMA.
- RoPE weight vectors folded into host-precomputed cos/sin tables
  (removes the negate + per-partition weight multiplies).
"""

import sys

sys.path.insert(0, "/opt/trn_rl_repo")

import numpy as np
import ml_dtypes
import concourse.bass as bass  # noqa: F401
import concourse.tile as tile
from concourse import mybir, bacc
from concourse.bass_utils import run_bass_kernel_spmd
from concourse.masks import make_identity

N_CORES = 8
D_IN = 2048
SEQ = 2048
N_HEADS = 32
N_KV = 8
HD = 64
HPC = N_HEADS // N_CORES  # 4 q heads per core
EPS = 1e-6
NEG = -1.0e9

F32 = mybir.dt.float32
F32R = mybir.dt.float32r
BF16 = mybir.dt.bfloat16
BF16_NP = ml_dtypes.bfloat16

KT_TILES = D_IN // 128  # 16 contraction tiles for projections
QB = 512  # q block
NQB = SEQ // QB  # 4
ROWS_PER_CORE = SEQ // N_CORES  # 256
AF = mybir.ActivationFunctionType


def _build():
    nc = bacc.Bacc(num_devices=N_CORES)

    xT = nc.dram_tensor("xT", [D_IN, SEQ], BF16, kind="ExternalInput")
    wq = nc.dram_tensor("wq", [D_IN, HPC * HD], BF16, kind="ExternalInput")
    wkv = nc.dram_tensor("wkv", [D_IN, 2 * HD], BF16, kind="ExternalInput")
    wo = nc.dram_tensor("wo", [D_IN, D_IN], BF16, kind="ExternalInput")
    coswq = nc.dram_tensor("coswq", [128, SEQ], F32, kind="ExternalInput")
    sinwq = nc.dram_tensor("sinwq", [128, SEQ], F32, kind="ExternalInput")
    coswk = nc.dram_tensor("coswk", [64, SEQ], F32, kind="ExternalInput")
    sinwk = nc.dram_tensor("sinwk", [64, SEQ], F32, kind="ExternalInput")
    tri = nc.dram_tensor("tri", [128, 128], F32, kind="ExternalInput")
    sel = nc.dram_tensor("sel", [4 * N_CORES, 2 * N_CORES, 128], F32R, kind="ExternalInput")
    onesblk_in = nc.dram_tensor("onesblk", [128, 128], F32R, kind="ExternalInput")

    out = nc.dram_tensor("out", [ROWS_PER_CORE, D_IN], F32, kind="ExternalOutput")

    with tile.TileContext(nc) as tc:
        with (
            tc.tile_pool(name="persist", bufs=1) as pers,
            tc.tile_pool(name="dram", bufs=1, space="DRAM") as dram,
        ):
            # ---- persistent SBUF preloads ----
            wq_sb = pers.tile([128, KT_TILES, HPC * HD], BF16)  # 1 MB
            nc.sync.dma_start(wq_sb[:], wq.rearrange("(ko p) m -> p ko m", p=128))
            wkv_sb = pers.tile([128, KT_TILES, 2 * HD], BF16)  # 0.5 MB
            nc.sync.dma_start(wkv_sb[:], wkv.rearrange("(ko p) m -> p ko m", p=128))
            cq_sb = pers.tile([128, SEQ], F32)
            sq_sb = pers.tile([128, SEQ], F32)
            ck_sb = pers.tile([64, SEQ], F32)
            sk_sb = pers.tile([64, SEQ], F32)
            nc.sync.dma_start(cq_sb[:], coswq[:])
            nc.sync.dma_start(sq_sb[:], sinwq[:])
            nc.sync.dma_start(ck_sb[:], coswk[:])
            nc.sync.dma_start(sk_sb[:], sinwk[:])
            tri_sb = pers.tile([128, 128], F32)
            nc.sync.dma_start(tri_sb[:], tri[:])
            sel_sb = pers.tile([4 * N_CORES, 2 * N_CORES, 128], F32R)
            nc.sync.dma_start(sel_sb[:], sel[:])
            onesblk = pers.tile([128, 128], F32R)
            nc.sync.dma_start(onesblk[:], onesblk_in[:])
            ident = pers.tile([128, 128], F32)
            make_identity(nc, ident[:])
            eps_sb = pers.tile([128, 1], F32)
            nc.vector.memset(eps_sb[:], EPS)
            # full Wo resident in bf16 (8 MB); loads overlap early compute
            wo_sb = pers.tile([128, KT_TILES, D_IN], BF16)
            nc.sync.dma_start(wo_sb[:], wo.rearrange("(ko p) m -> p ko m", p=128))

            # per-block persistent QKV (separate tiles per j for clean deps)
            qt = [pers.tile([64, HPC, QB], BF16, name=f"qt{j}") for j in range(NQB)]
            kt = [pers.tile([64, QB], BF16, name=f"kt{j}") for j in range(NQB)]
            vaug = [pers.tile([128, 4, HD + 1], BF16, name=f"va{j}") for j in range(NQB)]

            a2a_in = dram.tile([N_CORES, HPC * HD + HPC, ROWS_PER_CORE], F32)
            a2a_out = dram.tile([N_CORES, HPC * HD + HPC, ROWS_PER_CORE], F32)

            # ============ fused projections + attention, j-outer ============
            with (
                tc.tile_pool(name="xt", bufs=4) as xp,
                tc.tile_pool(name="acc", bufs=3, space="PSUM") as psA,
                tc.tile_pool(name="sc", bufs=2, space="PSUM") as psB,
                tc.tile_pool(name="pv", bufs=1, space="PSUM") as psPV,
                tc.tile_pool(name="work", bufs=2) as t1,
                tc.tile_pool(name="ptp", bufs=3) as ptp,
            ):
                for j in range(NQB):
                    sl = slice(QB * j, QB * j + QB)
                    # ---- projection chains for block j ----
                    acc = [
                        psA.tile([128, QB], F32, tag="acc", name=f"acc{i}_{j}")
                        for i in range(3)
                    ]
                    for k in range(KT_TILES):
                        xt = xp.tile([128, QB], BF16, tag="xt")
                        nc.sync.dma_start(xt[:], xT[128 * k : 128 * k + 128, sl])
                        st = k == 0
                        sp = k == KT_TILES - 1
                        nc.tensor.matmul(acc[0][:], wq_sb[:, k, 0:128], xt[:], start=st, stop=sp)
                        nc.tensor.matmul(acc[1][:], wq_sb[:, k, 128:256], xt[:], start=st, stop=sp)
                        nc.tensor.matmul(acc[2][:], wkv_sb[:, k, :], xt[:], start=st, stop=sp)

                    for idx in range(3):
                        is_kv = idx == 2
                        rows = slice(0, 64) if is_kv else slice(0, 128)
                        # evict PSUM early (frees bank; SBUF ops get DVE 2x)
                        rawsb = t1.tile([128, QB], F32, tag="rawsb")
                        nc.vector.tensor_copy(rawsb[:], acc[idx][:])
                        sq = t1.tile([128, QB], F32R, tag="sq")
                        nc.scalar.square(sq[:], rawsb[:])
                        psn = psB.tile([128, 2, QB], F32, tag="sc", name=f"psn{idx}_{j}")
                        nc.tensor.matmul(psn[:, 0, :], onesblk[:], sq[:], start=True, stop=True)
                        # rsqrt(var+eps) = exp(-0.5*ln(var+eps)); stays in the
                        # natural_log_exp ACT table set (no table switch)
                        lnv = t1.tile([128, QB], F32, tag="lnv")
                        nc.scalar.activation(
                            out=lnv[rows, :], in_=psn[rows, 0, :],
                            func=AF.Ln, bias=eps_sb[rows, :], scale=1.0 / HD,
                        )
                        rcp = t1.tile([128, QB], F32, tag="rcp")
                        nc.scalar.activation(
                            out=rcp[rows, :], in_=lnv[rows, :],
                            func=AF.Exp, scale=-0.5,
                        )
                        tn = t1.tile([128, QB], F32, tag="tn")
                        nc.vector.tensor_mul(tn[rows, :], rawsb[rows, :], rcp[rows, :])
                        # rot = tn shifted by 32 within each head (sign folded
                        # into the host-precomputed sin tables)
                        rot = t1.tile([128, QB], F32, tag="rot")
                        nh = 1 if is_kv else 2
                        for b in range(nh):
                            o = 64 * b
                            nc.vector.tensor_copy(rot[o : o + 32, :], tn[o + 32 : o + 64, :])
                            nc.vector.tensor_copy(rot[o + 32 : o + 64, :], tn[o : o + 32, :])
                        if is_kv:
                            tcs = t1.tile([64, QB], F32, tag="tcs")
                            nc.vector.tensor_mul(tcs[:], tn[0:64, :], ck_sb[:, sl])
                            nc.vector.tensor_mul(rot[0:64, :], rot[0:64, :], sk_sb[:, sl])
                            nc.vector.tensor_add(kt[j][:], tcs[:], rot[0:64, :])
                            # V rows: transpose to [token, hd] layout
                            vt = t1.tile([64, QB], F32, tag="vt")
                            nc.vector.tensor_copy(vt[:], rawsb[64:128, :])
                            for d in range(4):
                                psv = psB.tile([128, 2, QB], F32, tag="sc", name=f"psv{j}_{d}")
                                nc.tensor.transpose(
                                    psv[:, 0, 0:64],
                                    vt[:, 128 * d : 128 * d + 128],
                                    ident[0:64, 0:64],
                                )
                                nc.vector.tensor_copy(vaug[j][:, d, 0:HD], psv[:, 0, 0:64])
                                nc.vector.memset(vaug[j][:, d, HD : HD + 1], 1.0)
                        else:
                            tc2 = t1.tile([128, QB], F32, tag="tc2")
                            nc.vector.tensor_mul(tc2[:], tn[:], cq_sb[:, sl])
                            nc.vector.tensor_mul(rot[:], rot[:], sq_sb[:, sl])
                            for b in range(2):
                                nc.vector.tensor_add(
                                    qt[j][:, 2 * idx + b, :],
                                    tc2[64 * b : 64 * b + 64, :],
                                    rot[64 * b : 64 * b + 64, :],
                                )

                    # ---- attention for block j (kv tiles 0..4j+3) ----
                    for h in range(HPC):
                        pv = psPV.tile([128, QB], F32, tag="pv", name=f"pv{j}_{h}")
                        nfull = 4 * j
                        for g in range(0, nfull, 2):
                            sc = psB.tile([128, 2, QB], F32, tag="sc", name=f"sc{j}_{h}_{g}")
                            for u in range(2):
                                t = g + u
                                jj, d = t // 4, t % 4
                                nc.tensor.matmul(
                                    sc[:, u, :],
                                    kt[jj][:, 128 * d : 128 * d + 128],
                                    qt[j][:, h, :],
                                    start=True, stop=True,
                                )
                            pt = ptp.tile([128, 2, QB], BF16, tag="pt")
                            nc.scalar.activation(
                                out=pt[:, :, :], in_=sc[:, :, :],
                                func=AF.Exp, scale=0.125,
                            )
                            for u in range(2):
                                t = g + u
                                jj, d = t // 4, t % 4
                                nc.tensor.matmul(
                                    pv[0:65, :],
                                    vaug[jj][:, d, :],
                                    pt[:, u, :],
                                    start=(t == 0), stop=False,
                                )
                        for d in range(4):
                            t = 4 * j + d
                            n0 = 128 * d
                            sc = psB.tile([128, 2, QB], F32, tag="sc", name=f"scd{j}_{h}_{d}")
                            nc.tensor.matmul(
                                sc[:, 0, 0 : QB - n0],
                                kt[j][:, 128 * d : 128 * d + 128],
                                qt[j][:, h, n0:QB],
                                start=True, stop=True,
                            )
                            nc.vector.tensor_add(sc[:, 0, 0:128], sc[:, 0, 0:128], tri_sb[:])
                            pt = ptp.tile([128, 2, QB], BF16, tag="pt")
                            nc.scalar.activation(
                                out=pt[:, 0, 0 : QB - n0], in_=sc[:, 0, 0 : QB - n0],
                                func=AF.Exp, scale=0.125,
                            )
                            nc.tensor.matmul(
                                pv[0:65, n0:QB],
                                vaug[j][:, d, :],
                                pt[:, 0, 0 : QB - n0],
                                start=(t == 0), stop=(d == 3),
                            )
                        att = t1.tile([65, QB], F32, tag="att")
                        nc.vector.tensor_copy(att[:], pv[0:65, :])
                        for s in range(2):
                            shard = 2 * j + s
                            cs = slice(ROWS_PER_CORE * s, ROWS_PER_CORE * (s + 1))
                            nc.sync.dma_start(
                                a2a_in[shard, 64 * h : 64 * h + 64, :], att[0:64, cs]
                            )
                            nc.sync.dma_start(
                                a2a_in[shard, HPC * 64 + h, :], att[64:65, cs]
                            )

            # ================= AllToAll ==============================
            nc.gpsimd.collective_compute(
                "AllToAll",
                mybir.AluOpType.bypass,
                replica_groups=[list(range(N_CORES))],
                ins=[a2a_in[:].opt()],
                outs=[a2a_out[:].opt()],
            )

            # ================= out projection ========================
            with (
                tc.tile_pool(name="p5ps", bufs=2, space="PSUM") as psC,
                tc.tile_pool(name="p5bc", bufs=2, space="PSUM") as psD,
                tc.tile_pool(name="p5sb", bufs=4) as t5,
                tc.tile_pool(name="an", bufs=1) as anp,
            ):
                R = ROWS_PER_CORE
                dsb_raw = t5.tile([4 * N_CORES, R], F32, tag="denraw")
                for g in range(N_CORES):
                    nc.sync.dma_start(
                        dsb_raw[4 * g : 4 * g + 4, :],
                        a2a_out[g, HPC * 64 : HPC * 64 + 4, :],
                    )
                dsb_inv = t5.tile([4 * N_CORES, R], F32, tag="deninv")
                nc.vector.reciprocal_approx_fast(out=dsb_inv[:], in_=dsb_raw[:])
                dsb = t5.tile([4 * N_CORES, R], F32R, tag="den")
                nc.vector.tensor_copy(dsb[:], dsb_inv[:])

                an_sb = anp.tile([128, 2 * N_CORES, R], BF16)  # normalized attnT
                for g in range(N_CORES):
                    for half in range(2):
                        a_raw = t5.tile([128, R], F32, tag="araw")
                        nc.sync.dma_start(
                            a_raw[:], a2a_out[g, 128 * half : 128 * half + 128, :]
                        )
                        bc = psD.tile([128, R], F32, tag="bc")
                        nc.tensor.matmul(
                            bc[:], sel_sb[:, 2 * g + half, :], dsb[:],
                            start=True, stop=True,
                        )
                        nc.vector.tensor_mul(an_sb[:, 2 * g + half, :], a_raw[:], bc[:])

                NB_OUT = D_IN // 512  # 4
                for nb in range(NB_OUT):
                    osl = slice(512 * nb, 512 * nb + 512)
                    po = [
                        psC.tile([128, 512], F32, tag=f"o{q}", name=f"o{q}_{nb}")
                        for q in range(2)
                    ]
                    for gh in range(2 * N_CORES):
                        for q in range(2):
                            nc.tensor.matmul(
                                po[q][:],
                                an_sb[:, gh, 128 * q : 128 * q + 128],
                                wo_sb[:, gh, osl],
                                start=(gh == 0), stop=(gh == 2 * N_CORES - 1),
                            )
                    for q in range(2):
                        osb = t5.tile([128, 512], F32, tag="osb")
                        nc.vector.tensor_copy(osb[:], po[q][:])
                        nc.sync.dma_start(out[128 * q : 128 * q + 128, osl], osb[:])

    nc.compile()
    return nc


_NC_CACHE = None


def _get_nc():
    global _NC_CACHE
    if _NC_CACHE is None:
        _NC_CACHE = _build()
    return _NC_CACHE


def _make_in_maps(x, cos, sin, wq, wk, wv, wo, q_norm_w, k_norm_w):
    x = np.asarray(x, dtype=np.float32)
    cos = np.asarray(cos, dtype=np.float32)
    sin = np.asarray(sin, dtype=np.float32)
    wq = np.asarray(wq, dtype=np.float32)
    wk = np.asarray(wk, dtype=np.float32)
    wv = np.asarray(wv, dtype=np.float32)
    wo = np.asarray(wo, dtype=np.float32)
    qw = np.asarray(q_norm_w, dtype=np.float32)
    kw = np.asarray(k_norm_w, dtype=np.float32)

    xT = np.ascontiguousarray(x[0].T).astype(BF16_NP)  # [D_IN, SEQ]
    wo_b = np.ascontiguousarray(wo).astype(BF16_NP)

    # RoPE tables with the norm weights folded in. For x normalized (tn),
    # rope(w*x)[d] = x[d]*(w[d]c[d]) + x[(d+32)%64]*(sgn[d]*w[(d+32)%64]*s[d])
    cosT = cos.T  # [64, SEQ]
    sinT = sin.T
    sgn = np.where(np.arange(64) < 32, -1.0, 1.0).astype(np.float32)
    wrot_q = qw[(np.arange(64) + 32) % 64]
    wrot_k = kw[(np.arange(64) + 32) % 64]
    cq1 = cosT * qw[:, None]
    sq1 = sinT * (sgn * wrot_q)[:, None]
    coswq = np.ascontiguousarray(np.vstack([cq1, cq1]))  # [128, SEQ]
    sinwq = np.ascontiguousarray(np.vstack([sq1, sq1]))
    coswk = np.ascontiguousarray(cosT * kw[:, None])
    sinwk = np.ascontiguousarray(sinT * (sgn * wrot_k)[:, None])

    ii, jj = np.meshgrid(np.arange(128), np.arange(128), indexing="ij")
    tri = np.where(ii <= jj, 0.0, NEG).astype(np.float32)  # keep kv<=q
    onesblk = np.zeros((128, 128), np.float32)
    onesblk[0:64, 0:64] = 1.0
    onesblk[64:128, 64:128] = 1.0
    sel = np.zeros((4 * N_CORES, 2 * N_CORES, 128), np.float32)
    for g in range(N_CORES):
        for half in range(2):
            for m in range(128):
                sel[4 * g + 2 * half + m // 64, 2 * g + half, m] = 1.0

    in_maps = []
    for c in range(N_CORES):
        wq_c = np.ascontiguousarray(wq[:, 256 * c : 256 * c + 256]).astype(BF16_NP)
        wkv_c = np.ascontiguousarray(
            np.concatenate(
                [wk[:, 64 * c : 64 * c + 64], wv[:, 64 * c : 64 * c + 64]], axis=1
            )
        ).astype(BF16_NP)
        in_maps.append(
            {
                "xT": xT,
                "wq": wq_c,
                "wkv": wkv_c,
                "wo": wo_b,
                "coswq": coswq,
                "sinwq": sinwq,
                "coswk": coswk,
                "sinwk": sinwk,
                "tri": tri,
                "sel": sel,
                "onesblk": onesblk,
            }
        )
    return in_maps


def kernel(x, cos, sin, wq, wk, wv, wo, q_norm_w, k_norm_w):
    in_maps = _make_in_maps(x, cos, sin, wq, wk, wv, wo, q_norm_w, k_norm_w)
    nc = _get_nc()
    res = run_bass_kernel_spmd(nc, in_maps, core_ids=list(range(N_CORES)))
    rows = [res.results[c]["out"] for c in range(N_CORES)]
    full = np.concatenate(rows, axis=0)  # [SEQ, D_IN]
    return full.reshape(1, SEQ, D_IN).astype(np.float32)


# revision 10
# speedup vs baseline: 1.4088x; 1.1801x over previous
"""GQA FlashAttention (RMSNorm QK + RoPE, causal) on 8 TRN2 NeuronCores.

Sharding: tensor-parallel over heads (core c owns q-heads 4c..4c+3 and
kv-head c; the GQA group is fully local). Head-chunked AllToAlls re-shard
the attention output from head-parallel to row-parallel; each core then
computes its 256 output rows against the full Wo.

v3 vs v2:
- DMA queue separation: xt loads issue from the Sync sequencer; all
  attention-output stores, big preloads (wo), and phase-5 loads issue
  from the GpSimd sequencer. v2 had the att stores head-of-line blocking
  the next block's xt prefetches on the shared sync queue, serializing
  projections against attention.
- Square moved to DVE (v2 thrashed ACT tables 17x between the square/ln
  set picks; now only ln+exp remain, one natural_log_exp set).
- AllToAll split into 4 head-chunks: chunk h fires right after head h of
  the last q-block finishes, so chunks 0-2 transfer under the tail of
  attention and phase-5 prep (denominators, bc broadcasts, half-0
  out-proj accumulation) runs under chunks 2-3.
- Deeper xt prefetch (8 buffers).
"""

import sys

sys.path.insert(0, "/opt/trn_rl_repo")

import numpy as np
import ml_dtypes
import concourse.bass as bass  # noqa: F401
import concourse.tile as tile
from concourse import mybir, bacc
from concourse.bass_utils import run_bass_kernel_spmd
from concourse.masks import make_identity

N_CORES = 8
D_IN = 2048
SEQ = 2048
N_HEADS = 32
N_KV = 8
HD = 64
HPC = N_HEADS // N_CORES  # 4 q heads per core
EPS = 1e-6
NEG = -1.0e9

F32 = mybir.dt.float32
F32R = mybir.dt.float32r
BF16 = mybir.dt.bfloat16
BF16_NP = ml_dtypes.bfloat16

KT_TILES = D_IN // 128
QB = 512
NQB = SEQ // QB  # 4
ROWS_PER_CORE = SEQ // N_CORES  # 256
AF = mybir.ActivationFunctionType


def _build():
    nc = bacc.Bacc(num_devices=N_CORES)

    xT = nc.dram_tensor("xT", [D_IN, SEQ], BF16, kind="ExternalInput")
    wq = nc.dram_tensor("wq", [D_IN, HPC * HD], BF16, kind="ExternalInput")
    wkv = nc.dram_tensor("wkv", [D_IN, 2 * HD], BF16, kind="ExternalInput")
    wo = nc.dram_tensor("wo", [D_IN, D_IN], BF16, kind="ExternalInput")
    coswq = nc.dram_tensor("coswq", [128, SEQ], F32, kind="ExternalInput")
    sinwq = nc.dram_tensor("sinwq", [128, SEQ], F32, kind="ExternalInput")
    coswk = nc.dram_tensor("coswk", [64, SEQ], F32, kind="ExternalInput")
    sinwk = nc.dram_tensor("sinwk", [64, SEQ], F32, kind="ExternalInput")
    tri = nc.dram_tensor("tri", [128, 128], F32, kind="ExternalInput")
    sel16 = nc.dram_tensor("sel16", [2 * N_CORES, 2 * N_CORES, 128], F32R, kind="ExternalInput")
    onesblk_in = nc.dram_tensor("onesblk", [128, 128], F32R, kind="ExternalInput")

    out = nc.dram_tensor("out", [ROWS_PER_CORE, D_IN], F32, kind="ExternalOutput")

    with tile.TileContext(nc) as tc:
        with (
            tc.tile_pool(name="persist", bufs=1) as pers,
            tc.tile_pool(name="dram", bufs=1, space="DRAM") as dram,
        ):
            # ---- persistent SBUF preloads ----
            # big / late-needed tensors go via the GpSimd queue so they do
            # not delay the first xt loads on the Sync queue
            wo_sb = pers.tile([128, KT_TILES, D_IN], BF16)  # 8 MB
            nc.scalar.dma_start(wo_sb[:], wo.rearrange("(ko p) m -> p ko m", p=128))
            sel_sb = pers.tile([2 * N_CORES, 2 * N_CORES, 128], F32R)
            nc.scalar.dma_start(sel_sb[:], sel16[:])

            wq_sb = pers.tile([128, KT_TILES, HPC * HD], BF16)  # 1 MB
            nc.sync.dma_start(wq_sb[:], wq.rearrange("(ko p) m -> p ko m", p=128))
            wkv_sb = pers.tile([128, KT_TILES, 2 * HD], BF16)  # 0.5 MB
            nc.sync.dma_start(wkv_sb[:], wkv.rearrange("(ko p) m -> p ko m", p=128))
            cq_sb = pers.tile([128, SEQ], F32)
            sq_sb = pers.tile([128, SEQ], F32)
            ck_sb = pers.tile([64, SEQ], F32)
            sk_sb = pers.tile([64, SEQ], F32)
            nc.sync.dma_start(cq_sb[:], coswq[:])
            nc.sync.dma_start(sq_sb[:], sinwq[:])
            nc.sync.dma_start(ck_sb[:], coswk[:])
            nc.sync.dma_start(sk_sb[:], sinwk[:])
            tri_sb = pers.tile([128, 128], F32)
            nc.sync.dma_start(tri_sb[:], tri[:])
            onesblk = pers.tile([128, 128], F32R)
            nc.sync.dma_start(onesblk[:], onesblk_in[:])
            ident = pers.tile([128, 128], F32)
            make_identity(nc, ident[:])
            eps_sb = pers.tile([128, 1], F32)
            nc.vector.memset(eps_sb[:], EPS)

            qt = [pers.tile([64, HPC, QB], BF16, name=f"qt{j}") for j in range(NQB)]
            kt = [pers.tile([64, QB], BF16, name=f"kt{j}") for j in range(NQB)]
            vaug = [pers.tile([128, 4, HD + 1], BF16, name=f"va{j}") for j in range(NQB)]

            a2a_in = [
                dram.tile([N_CORES, HD + 1, ROWS_PER_CORE], F32, name=f"a2ai{h}")
                for h in range(HPC)
            ]
            a2a_out = [
                dram.tile([N_CORES, HD + 1, ROWS_PER_CORE], F32, name=f"a2ao{h}")
                for h in range(HPC)
            ]

            # ============ fused projections + attention, j-outer ============
            with (
                tc.tile_pool(name="xt", bufs=8) as xp,
                tc.tile_pool(name="acc", bufs=3, space="PSUM") as psA,
                tc.tile_pool(name="sc", bufs=2, space="PSUM") as psB,
                tc.tile_pool(name="pv", bufs=1, space="PSUM") as psPV,
                tc.tile_pool(name="work", bufs=2) as t1,
                tc.tile_pool(name="ptp", bufs=3) as ptp,
            ):
                for j in range(NQB):
                    sl = slice(QB * j, QB * j + QB)
                    # ---- projection chains for block j ----
                    acc = [
                        psA.tile([128, QB], F32, tag="acc", name=f"acc{i}_{j}")
                        for i in range(3)
                    ]
                    for k in range(KT_TILES):
                        xt = xp.tile([128, QB], BF16, tag="xt")
                        nc.sync.dma_start(xt[:], xT[128 * k : 128 * k + 128, sl])
                        st = k == 0
                        sp = k == KT_TILES - 1
                        nc.tensor.matmul(acc[0][:], wq_sb[:, k, 0:128], xt[:], start=st, stop=sp)
                        nc.tensor.matmul(acc[1][:], wq_sb[:, k, 128:256], xt[:], start=st, stop=sp)
                        nc.tensor.matmul(acc[2][:], wkv_sb[:, k, :], xt[:], start=st, stop=sp)

                    for idx in range(3):
                        is_kv = idx == 2
                        rows = slice(0, 64) if is_kv else slice(0, 128)
                        rawsb = t1.tile([128, QB], F32, tag="rawsb")
                        nc.vector.tensor_copy(rawsb[:], acc[idx][:])
                        sq = t1.tile([128, QB], F32R, tag="sq")
                        nc.vector.tensor_mul(sq[:], rawsb[:], rawsb[:])
                        psn = psB.tile([128, 2, QB], F32, tag="sc", name=f"psn{idx}_{j}")
                        nc.tensor.matmul(psn[:, 0, :], onesblk[:], sq[:], start=True, stop=True)
                        # rsqrt(var+eps) = exp(-0.5*ln(var+eps)); ln+exp live
                        # in one ACT table set (no switches kernel-wide)
                        lnv = t1.tile([128, QB], F32, tag="lnv")
                        nc.scalar.activation(
                            out=lnv[rows, :], in_=psn[rows, 0, :],
                            func=AF.Ln, bias=eps_sb[rows, :], scale=1.0 / HD,
                        )
                        rcp = t1.tile([128, QB], F32, tag="rcp")
                        nc.scalar.activation(
                            out=rcp[rows, :], in_=lnv[rows, :],
                            func=AF.Exp, scale=-0.5,
                        )
                        tn = t1.tile([128, QB], F32, tag="tn")
                        nc.vector.tensor_mul(tn[rows, :], rawsb[rows, :], rcp[rows, :])
                        rot = t1.tile([128, QB], F32, tag="rot")
                        nh = 1 if is_kv else 2
                        for b in range(nh):
                            o = 64 * b
                            nc.vector.tensor_copy(rot[o : o + 32, :], tn[o + 32 : o + 64, :])
                            nc.vector.tensor_copy(rot[o + 32 : o + 64, :], tn[o : o + 32, :])
                        if is_kv:
                            tcs = t1.tile([64, QB], F32, tag="tcs")
                            nc.vector.tensor_mul(tcs[:], tn[0:64, :], ck_sb[:, sl])
                            nc.vector.tensor_mul(rot[0:64, :], rot[0:64, :], sk_sb[:, sl])
                            nc.vector.tensor_add(kt[j][:], tcs[:], rot[0:64, :])
                            vt = t1.tile([64, QB], F32, tag="vt")
                            nc.vector.tensor_copy(vt[:], rawsb[64:128, :])
                            for d in range(4):
                                psv = psB.tile([128, 2, QB], F32, tag="sc", name=f"psv{j}_{d}")
                                nc.tensor.transpose(
                                    psv[:, 0, 0:64],
                                    vt[:, 128 * d : 128 * d + 128],
                                    ident[0:64, 0:64],
                                )
                                nc.vector.tensor_copy(vaug[j][:, d, 0:HD], psv[:, 0, 0:64])
                                nc.vector.memset(vaug[j][:, d, HD : HD + 1], 1.0)
                        else:
                            tc2 = t1.tile([128, QB], F32, tag="tc2")
                            nc.vector.tensor_mul(tc2[:], tn[:], cq_sb[:, sl])
                            nc.vector.tensor_mul(rot[:], rot[:], sq_sb[:, sl])
                            for b in range(2):
                                nc.vector.tensor_add(
                                    qt[j][:, 2 * idx + b, :],
                                    tc2[64 * b : 64 * b + 64, :],
                                    rot[64 * b : 64 * b + 64, :],
                                )

                    # ---- attention for block j (kv tiles 0..4j+3) ----
                    for h in range(HPC):
                        pv = psPV.tile([128, QB], F32, tag="pv", name=f"pv{j}_{h}")
                        nfull = 4 * j
                        for g in range(0, nfull, 2):
                            sc = psB.tile([128, 2, QB], F32, tag="sc", name=f"sc{j}_{h}_{g}")
                            for u in range(2):
                                t = g + u
                                jj, d = t // 4, t % 4
                                nc.tensor.matmul(
                                    sc[:, u, :],
                                    kt[jj][:, 128 * d : 128 * d + 128],
                                    qt[j][:, h, :],
                                    start=True, stop=True,
                                )
                            pt = ptp.tile([128, 2, QB], BF16, tag="pt")
                            nc.scalar.activation(
                                out=pt[:, :, :], in_=sc[:, :, :],
                                func=AF.Exp, scale=0.125,
                            )
                            for u in range(2):
                                t = g + u
                                jj, d = t // 4, t % 4
                                nc.tensor.matmul(
                                    pv[0:65, :],
                                    vaug[jj][:, d, :],
                                    pt[:, u, :],
                                    start=(t == 0), stop=False,
                                )
                        for d in range(4):
                            t = 4 * j + d
                            n0 = 128 * d
                            sc = psB.tile([128, 2, QB], F32, tag="sc", name=f"scd{j}_{h}_{d}")
                            nc.tensor.matmul(
                                sc[:, 0, 0 : QB - n0],
                                kt[j][:, 128 * d : 128 * d + 128],
                                qt[j][:, h, n0:QB],
                                start=True, stop=True,
                            )
                            nc.vector.tensor_add(sc[:, 0, 0:128], sc[:, 0, 0:128], tri_sb[:])
                            pt = ptp.tile([128, 2, QB], BF16, tag="pt")
                            nc.scalar.activation(
                                out=pt[:, 0, 0 : QB - n0], in_=sc[:, 0, 0 : QB - n0],
                                func=AF.Exp, scale=0.125,
                            )
                            nc.tensor.matmul(
                                pv[0:65, n0:QB],
                                vaug[j][:, d, :],
                                pt[:, 0, 0 : QB - n0],
                                start=(t == 0), stop=(d == 3),
                            )
                        att = t1.tile([65, QB], F32, tag="att")
                        nc.vector.tensor_copy(att[:], pv[0:65, :])
                        for s in range(2):
                            shard = 2 * j + s
                            cs = slice(ROWS_PER_CORE * s, ROWS_PER_CORE * (s + 1))
                            nc.gpsimd.dma_start(
                                a2a_in[h][shard, 0:64, :], att[0:64, cs]
                            )
                            nc.gpsimd.dma_start(
                                a2a_in[h][shard, 64, :], att[64:65, cs]
                            )

            # ============ head-chunked AllToAlls =====================
            for h in range(HPC):
                nc.gpsimd.collective_compute(
                    "AllToAll",
                    mybir.AluOpType.bypass,
                    replica_groups=[list(range(N_CORES))],
                    ins=[a2a_in[h][:].opt()],
                    outs=[a2a_out[h][:].opt()],
                )

            # ================= out projection ========================
            with (
                tc.tile_pool(name="p5ps", bufs=2, space="PSUM") as psC,
                tc.tile_pool(name="p5bc", bufs=2, space="PSUM") as psD,
                tc.tile_pool(name="p5sb", bufs=4) as t5,
                tc.tile_pool(name="an", bufs=1) as anp,
            ):
                R = ROWS_PER_CORE
                # denominators, one 16-partition tile per chunk pair
                # (base partition 0 must match sel_sb for the bc matmul)
                dsb_raw = [
                    t5.tile([2 * N_CORES, R], F32, tag=f"denraw{i}", name=f"denraw{i}")
                    for i in range(2)
                ]
                dsb_inv = [
                    t5.tile([2 * N_CORES, R], F32, tag=f"deninv{i}", name=f"deninv{i}")
                    for i in range(2)
                ]
                dsb = [
                    t5.tile([2 * N_CORES, R], F32R, tag=f"den{i}", name=f"den{i}")
                    for i in range(2)
                ]
                an_sb = anp.tile([128, 2 * N_CORES, R], BF16)

                def den_chunk(h):
                    hf, rs = h // 2, slice(8 * (h % 2), 8 * (h % 2) + 8)
                    nc.sync.dma_start(dsb_raw[hf][rs, :], a2a_out[h][:, 64, :])

                def an_half(half):
                    nc.vector.reciprocal_approx_fast(
                        out=dsb_inv[half][:, :], in_=dsb_raw[half][:, :]
                    )
                    nc.vector.tensor_copy(dsb[half][:, :], dsb_inv[half][:, :])
                    for g in range(N_CORES):
                        a_raw = t5.tile([128, R], F32, tag="araw")
                        nc.sync.dma_start(
                            a_raw[0:64, :], a2a_out[2 * half][g, 0:64, :]
                        )
                        nc.sync.dma_start(
                            a_raw[64:128, :], a2a_out[2 * half + 1][g, 0:64, :]
                        )
                        bc = psD.tile([128, R], F32, tag="bc")
                        nc.tensor.matmul(
                            bc[:],
                            sel_sb[:, 2 * g + half, :],
                            dsb[half][:, :],
                            start=True, stop=True,
                        )
                        nc.vector.tensor_mul(an_sb[:, 2 * g + half, :], a_raw[:], bc[:])

                den_chunk(0)
                den_chunk(1)
                an_half(0)
                den_chunk(2)
                den_chunk(3)
                an_half(1)

                NB_OUT = D_IN // 512  # 4
                for nb in range(NB_OUT):
                    osl = slice(512 * nb, 512 * nb + 512)
                    po = [
                        psC.tile([128, 512], F32, tag=f"o{q}", name=f"o{q}_{nb}")
                        for q in range(2)
                    ]
                    for half in range(2):
                        for g in range(N_CORES):
                            gh = 2 * g + half
                            first = half == 0 and g == 0
                            last = half == 1 and g == N_CORES - 1
                            for q in range(2):
                                nc.tensor.matmul(
                                    po[q][:],
                                    an_sb[:, gh, 128 * q : 128 * q + 128],
                                    wo_sb[:, gh, osl],
                                    start=first, stop=last,
                                )
                    for q in range(2):
                        osb = t5.tile([128, 512], F32, tag="osb")
                        nc.vector.tensor_copy(osb[:], po[q][:])
                        nc.sync.dma_start(out[128 * q : 128 * q + 128, osl], osb[:])

    nc.compile()
    return nc


_NC_CACHE = None


def _get_nc():
    global _NC_CACHE
    if _NC_CACHE is None:
        _NC_CACHE = _build()
    return _NC_CACHE


def _make_in_maps(x, cos, sin, wq, wk, wv, wo, q_norm_w, k_norm_w):
    x = np.asarray(x, dtype=np.float32)
    cos = np.asarray(cos, dtype=np.float32)
    sin = np.asarray(sin, dtype=np.float32)
    wq = np.asarray(wq, dtype=np.float32)
    wk = np.asarray(wk, dtype=np.float32)
    wv = np.asarray(wv, dtype=np.float32)
    wo = np.asarray(wo, dtype=np.float32)
    qw = np.asarray(q_norm_w, dtype=np.float32)
    kw = np.asarray(k_norm_w, dtype=np.float32)

    xT = np.ascontiguousarray(x[0].T).astype(BF16_NP)
    wo_b = np.ascontiguousarray(wo).astype(BF16_NP)

    cosT = cos.T  # [64, SEQ]
    sinT = sin.T
    sgn = np.where(np.arange(64) < 32, -1.0, 1.0).astype(np.float32)
    wrot_q = qw[(np.arange(64) + 32) % 64]
    wrot_k = kw[(np.arange(64) + 32) % 64]
    cq1 = cosT * qw[:, None]
    sq1 = sinT * (sgn * wrot_q)[:, None]
    coswq = np.ascontiguousarray(np.vstack([cq1, cq1]))
    sinwq = np.ascontiguousarray(np.vstack([sq1, sq1]))
    coswk = np.ascontiguousarray(cosT * kw[:, None])
    sinwk = np.ascontiguousarray(sinT * (sgn * wrot_k)[:, None])

    ii, jj = np.meshgrid(np.arange(128), np.arange(128), indexing="ij")
    tri = np.where(ii <= jj, 0.0, NEG).astype(np.float32)
    onesblk = np.zeros((128, 128), np.float32)
    onesblk[0:64, 0:64] = 1.0
    onesblk[64:128, 64:128] = 1.0
    # bc broadcast selector vs chunk-major dsb halves: for an slice
    # (g, half), partition m needs den row 8*(m//64) + g of dsb half
    sel16 = np.zeros((2 * N_CORES, 2 * N_CORES, 128), np.float32)
    for g in range(N_CORES):
        for half in range(2):
            for m in range(128):
                sel16[8 * (m // 64) + g, 2 * g + half, m] = 1.0

    in_maps = []
    for c in range(N_CORES):
        wq_c = np.ascontiguousarray(wq[:, 256 * c : 256 * c + 256]).astype(BF16_NP)
        wkv_c = np.ascontiguousarray(
            np.concatenate(
                [wk[:, 64 * c : 64 * c + 64], wv[:, 64 * c : 64 * c + 64]], axis=1
            )
        ).astype(BF16_NP)
        in_maps.append(
            {
                "xT": xT,
                "wq": wq_c,
                "wkv": wkv_c,
                "wo": wo_b,
                "coswq": coswq,
                "sinwq": sinwq,
                "coswk": coswk,
                "sinwk": sinwk,
                "tri": tri,
                "sel16": sel16,
                "onesblk": onesblk,
            }
        )
    return in_maps


def kernel(x, cos, sin, wq, wk, wv, wo, q_norm_w, k_norm_w):
    in_maps = _make_in_maps(x, cos, sin, wq, wk, wv, wo, q_norm_w, k_norm_w)
    nc = _get_nc()
    res = run_bass_kernel_spmd(nc, in_maps, core_ids=list(range(N_CORES)))
    rows = [res.results[c]["out"] for c in range(N_CORES)]
    full = np.concatenate(rows, axis=0)  # [SEQ, D_IN]
    return full.reshape(1, SEQ, D_IN).astype(np.float32)


# revision 11
# speedup vs baseline: 1.4323x; 1.0167x over previous
"""GQA FlashAttention (RMSNorm QK + RoPE, causal) on 8 TRN2 NeuronCores.

Sharding: tensor-parallel over heads (core c owns q-heads 4c..4c+3 and
kv-head c; the GQA group is fully local). Head-chunked AllToAlls re-shard
the attention output from head-parallel to row-parallel; each core then
computes its 256 output rows against the full Wo.

v4 vs v3:
- Software pipelining at emission: attention for block j-1 is emitted
  interleaved (unit by unit) with the projection chains for block j, so
  the PE instruction stream alternates dependency-stalled attention
  matmuls with always-ready projection matmuls. v3's static PE order ran
  whole phases back to back and head-of-line blocked on the exp chain,
  starving the PE (and HAM-throttling it to 1.2 GHz).
- ACT tables pinned to natural_log_exp_and_others (serves both the
  rmsnorm ln/exp and the softmax exp) - v3 reloaded tables 17x.
- Weights pre-transposed on host to the [partition, ktile, m] SBUF
  layout: preload DMAs are contiguous and issue fast.
- Preloads spread across queues: wq/wkv on Sync (gate the first matmul),
  rope tables etc on GpSimd, wo/sel on Scalar.
"""

import sys

sys.path.insert(0, "/opt/trn_rl_repo")

import numpy as np
import ml_dtypes
import concourse.bass as bass  # noqa: F401
import concourse.tile as tile
import concourse.bacc as bacc_mod
from concourse import mybir, bacc
from concourse.bass_utils import run_bass_kernel_spmd
from concourse.hw_specs import get_activation_tables as _orig_get_tables
from concourse.masks import make_identity

N_CORES = 8
D_IN = 2048
SEQ = 2048
N_HEADS = 32
N_KV = 8
HD = 64
HPC = N_HEADS // N_CORES  # 4 q heads per core
EPS = 1e-6
NEG = -1.0e9

F32 = mybir.dt.float32
F32R = mybir.dt.float32r
BF16 = mybir.dt.bfloat16
BF16_NP = ml_dtypes.bfloat16

KT_TILES = D_IN // 128
QB = 512
NQB = SEQ // QB  # 4
ROWS_PER_CORE = SEQ // N_CORES  # 256
AF = mybir.ActivationFunctionType

_ONE_TABLE = "natural_log_exp_and_others"


def _pinned_tables(arch):
    """Only natural_log_exp_and_others is selectable (it has ln AND exp);
    other entries keep their position (ids index the original list) but
    serve no functions, so the table-load pass never alternates sets."""
    tabs = _orig_get_tables(arch)
    return {n: (fs if n == _ONE_TABLE else set()) for n, fs in tabs.items()}


def _build():
    bacc_mod.get_activation_tables = _pinned_tables
    nc = bacc.Bacc(num_devices=N_CORES)

    xT = nc.dram_tensor("xT", [D_IN, SEQ], BF16, kind="ExternalInput")
    wq = nc.dram_tensor("wq", [128, KT_TILES, HPC * HD], BF16, kind="ExternalInput")
    wkv = nc.dram_tensor("wkv", [128, KT_TILES, 2 * HD], BF16, kind="ExternalInput")
    wo = nc.dram_tensor("wo", [128, KT_TILES, D_IN], BF16, kind="ExternalInput")
    coswq = nc.dram_tensor("coswq", [128, SEQ], F32, kind="ExternalInput")
    sinwq = nc.dram_tensor("sinwq", [128, SEQ], F32, kind="ExternalInput")
    coswk = nc.dram_tensor("coswk", [64, SEQ], F32, kind="ExternalInput")
    sinwk = nc.dram_tensor("sinwk", [64, SEQ], F32, kind="ExternalInput")
    tri = nc.dram_tensor("tri", [128, 128], F32, kind="ExternalInput")
    sel16 = nc.dram_tensor("sel16", [2 * N_CORES, 2 * N_CORES, 128], F32R, kind="ExternalInput")
    onesblk_in = nc.dram_tensor("onesblk", [128, 128], F32R, kind="ExternalInput")

    out = nc.dram_tensor("out", [ROWS_PER_CORE, D_IN], F32, kind="ExternalOutput")

    with tile.TileContext(nc) as tc:
        with (
            tc.tile_pool(name="persist", bufs=1) as pers,
            tc.tile_pool(name="dram", bufs=1, space="DRAM") as dram,
        ):
            # ---- persistent SBUF preloads (contiguous, host-transposed) ----
            wq_sb = pers.tile([128, KT_TILES, HPC * HD], BF16)
            nc.sync.dma_start(wq_sb[:], wq[:])
            wkv_sb = pers.tile([128, KT_TILES, 2 * HD], BF16)
            nc.sync.dma_start(wkv_sb[:], wkv[:])

            cq_sb = pers.tile([128, SEQ], F32)
            sq_sb = pers.tile([128, SEQ], F32)
            ck_sb = pers.tile([64, SEQ], F32)
            sk_sb = pers.tile([64, SEQ], F32)
            nc.gpsimd.dma_start(cq_sb[:], coswq[:])
            nc.gpsimd.dma_start(sq_sb[:], sinwq[:])
            nc.gpsimd.dma_start(ck_sb[:], coswk[:])
            nc.gpsimd.dma_start(sk_sb[:], sinwk[:])
            tri_sb = pers.tile([128, 128], F32)
            nc.gpsimd.dma_start(tri_sb[:], tri[:])
            onesblk = pers.tile([128, 128], F32R)
            nc.gpsimd.dma_start(onesblk[:], onesblk_in[:])

            wo_sb = pers.tile([128, KT_TILES, D_IN], BF16)  # 8 MB
            nc.scalar.dma_start(wo_sb[:], wo[:])
            sel_sb = pers.tile([2 * N_CORES, 2 * N_CORES, 128], F32R)
            nc.scalar.dma_start(sel_sb[:], sel16[:])

            ident = pers.tile([128, 128], F32)
            make_identity(nc, ident[:])
            eps_sb = pers.tile([128, 1], F32)
            nc.vector.memset(eps_sb[:], EPS)

            qt = [pers.tile([64, HPC, QB], BF16, name=f"qt{j}") for j in range(NQB)]
            kt = [pers.tile([64, QB], BF16, name=f"kt{j}") for j in range(NQB)]
            vaug = [pers.tile([128, 4, HD + 1], BF16, name=f"va{j}") for j in range(NQB)]

            a2a_in = [
                dram.tile([N_CORES, HD + 1, ROWS_PER_CORE], F32, name=f"a2ai{h}")
                for h in range(HPC)
            ]
            a2a_out = [
                dram.tile([N_CORES, HD + 1, ROWS_PER_CORE], F32, name=f"a2ao{h}")
                for h in range(HPC)
            ]

            # ============ fused projections + attention ====================
            with (
                tc.tile_pool(name="xt", bufs=8) as xp,
                tc.tile_pool(name="acc", bufs=3, space="PSUM") as psA,
                tc.tile_pool(name="sc", bufs=2, space="PSUM") as psB,
                tc.tile_pool(name="pv", bufs=1, space="PSUM") as psPV,
                tc.tile_pool(name="work", bufs=2) as t1,
                tc.tile_pool(name="ptp", bufs=3) as ptp,
            ):

                def emit_proj(j):
                    """Generator: projection + norm + rope for block j,
                    yielding after each small unit of PE work."""
                    sl = slice(QB * j, QB * j + QB)
                    acc = [
                        psA.tile([128, QB], F32, tag="acc", name=f"acc{i}_{j}")
                        for i in range(3)
                    ]
                    for k in range(KT_TILES):
                        xt = xp.tile([128, QB], BF16, tag="xt", name=f"xt{j}_{k}")
                        nc.sync.dma_start(xt[:], xT[128 * k : 128 * k + 128, sl])
                        st = k == 0
                        sp = k == KT_TILES - 1
                        nc.tensor.matmul(acc[0][:], wq_sb[:, k, 0:128], xt[:], start=st, stop=sp)
                        nc.tensor.matmul(acc[1][:], wq_sb[:, k, 128:256], xt[:], start=st, stop=sp)
                        nc.tensor.matmul(acc[2][:], wkv_sb[:, k, :], xt[:], start=st, stop=sp)
                        if k % 2 == 1:
                            yield
                    for idx in range(3):
                        is_kv = idx == 2
                        rows = slice(0, 64) if is_kv else slice(0, 128)
                        rawsb = t1.tile([128, QB], F32, tag="rawsb")
                        nc.vector.tensor_copy(rawsb[:], acc[idx][:])
                        sq = t1.tile([128, QB], F32R, tag="sq")
                        nc.vector.tensor_mul(sq[:], rawsb[:], rawsb[:])
                        psn = psB.tile([128, 2, QB], F32, tag="sc", name=f"psn{idx}_{j}")
                        nc.tensor.matmul(psn[:, 0, :], onesblk[:], sq[:], start=True, stop=True)
                        lnv = t1.tile([128, QB], F32, tag="lnv")
                        nc.scalar.activation(
                            out=lnv[rows, :], in_=psn[rows, 0, :],
                            func=AF.Ln, bias=eps_sb[rows, :], scale=1.0 / HD,
                        )
                        rcp = t1.tile([128, QB], F32, tag="rcp")
                        nc.scalar.activation(
                            out=rcp[rows, :], in_=lnv[rows, :],
                            func=AF.Exp, scale=-0.5,
                        )
                        yield
                        tn = t1.tile([128, QB], F32, tag="tn")
                        nc.vector.tensor_mul(tn[rows, :], rawsb[rows, :], rcp[rows, :])
                        rot = t1.tile([128, QB], F32, tag="rot")
                        nh = 1 if is_kv else 2
                        for b in range(nh):
                            o = 64 * b
                            nc.vector.tensor_copy(rot[o : o + 32, :], tn[o + 32 : o + 64, :])
                            nc.vector.tensor_copy(rot[o + 32 : o + 64, :], tn[o : o + 32, :])
                        if is_kv:
                            tcs = t1.tile([64, QB], F32, tag="tcs")
                            nc.vector.tensor_mul(tcs[:], tn[0:64, :], ck_sb[:, sl])
                            nc.vector.tensor_mul(rot[0:64, :], rot[0:64, :], sk_sb[:, sl])
                            nc.vector.tensor_add(kt[j][:], tcs[:], rot[0:64, :])
                            vt = t1.tile([64, QB], F32, tag="vt")
                            nc.vector.tensor_copy(vt[:], rawsb[64:128, :])
                            for d in range(4):
                                psv = psB.tile([128, 2, QB], F32, tag="sc", name=f"psv{j}_{d}")
                                nc.tensor.transpose(
                                    psv[:, 0, 0:64],
                                    vt[:, 128 * d : 128 * d + 128],
                                    ident[0:64, 0:64],
                                )
                                nc.vector.tensor_copy(vaug[j][:, d, 0:HD], psv[:, 0, 0:64])
                                nc.vector.memset(vaug[j][:, d, HD : HD + 1], 1.0)
                                if d == 1:
                                    yield
                        else:
                            tc2 = t1.tile([128, QB], F32, tag="tc2")
                            nc.vector.tensor_mul(tc2[:], tn[:], cq_sb[:, sl])
                            nc.vector.tensor_mul(rot[:], rot[:], sq_sb[:, sl])
                            for b in range(2):
                                nc.vector.tensor_add(
                                    qt[j][:, 2 * idx + b, :],
                                    tc2[64 * b : 64 * b + 64, :],
                                    rot[64 * b : 64 * b + 64, :],
                                )
                        yield

                def emit_att(j):
                    """Generator: attention for block j, yielding after each
                    scores/exp/PV group."""
                    for h in range(HPC):
                        pv = psPV.tile([128, QB], F32, tag="pv", name=f"pv{j}_{h}")
                        nfull = 4 * j
                        for g in range(0, nfull, 2):
                            sc = psB.tile([128, 2, QB], F32, tag="sc", name=f"sc{j}_{h}_{g}")
                            for u in range(2):
                                t = g + u
                                jj, d = t // 4, t % 4
                                nc.tensor.matmul(
                                    sc[:, u, :],
                                    kt[jj][:, 128 * d : 128 * d + 128],
                                    qt[j][:, h, :],
                                    start=True, stop=True,
                                )
                            pt = ptp.tile([128, 2, QB], BF16, tag="pt")
                            nc.scalar.activation(
                                out=pt[:, :, :], in_=sc[:, :, :],
                                func=AF.Exp, scale=0.125,
                            )
                            for u in range(2):
                                t = g + u
                                jj, d = t // 4, t % 4
                                nc.tensor.matmul(
                                    pv[0:65, :],
                                    vaug[jj][:, d, :],
                                    pt[:, u, :],
                                    start=(t == 0), stop=False,
                                )
                            yield
                        for d in range(4):
                            t = 4 * j + d
                            n0 = 128 * d
                            sc = psB.tile([128, 2, QB], F32, tag="sc", name=f"scd{j}_{h}_{d}")
                            nc.tensor.matmul(
                                sc[:, 0, 0 : QB - n0],
                                kt[j][:, 128 * d : 128 * d + 128],
                                qt[j][:, h, n0:QB],
                                start=True, stop=True,
                            )
                            nc.vector.tensor_add(sc[:, 0, 0:128], sc[:, 0, 0:128], tri_sb[:])
                            pt = ptp.tile([128, 2, QB], BF16, tag="pt")
                            nc.scalar.activation(
                                out=pt[:, 0, 0 : QB - n0], in_=sc[:, 0, 0 : QB - n0],
                                func=AF.Exp, scale=0.125,
                            )
                            nc.tensor.matmul(
                                pv[0:65, n0:QB],
                                vaug[j][:, d, :],
                                pt[:, 0, 0 : QB - n0],
                                start=(t == 0), stop=(d == 3),
                            )
                            if d % 2 == 1:
                                yield
                        att = t1.tile([65, QB], F32, tag="att")
                        nc.vector.tensor_copy(att[:], pv[0:65, :])
                        for s in range(2):
                            shard = 2 * j + s
                            cs = slice(ROWS_PER_CORE * s, ROWS_PER_CORE * (s + 1))
                            nc.gpsimd.dma_start(
                                a2a_in[h][shard, 0:64, :], att[0:64, cs]
                            )
                            nc.gpsimd.dma_start(
                                a2a_in[h][shard, 64, :], att[64:65, cs]
                            )
                        yield h  # head h of block j fully emitted

                def drive(gen):
                    for _ in gen:
                        pass

                def interleave(att_gen, proj_gen, att_per_proj=2):
                    att_done = proj_done = False
                    while not (att_done and proj_done):
                        for _ in range(att_per_proj):
                            if not att_done:
                                att_done = next(att_gen, "END") == "END"
                        if not proj_done:
                            proj_done = next(proj_gen, "END") == "END"

                # j=0 projections run alone, then att(j-1) overlaps proj(j)
                drive(emit_proj(0))
                interleave(emit_att(0), emit_proj(1), att_per_proj=1)
                interleave(emit_att(1), emit_proj(2), att_per_proj=2)
                interleave(emit_att(2), emit_proj(3), att_per_proj=2)
                # last block's attention: fire each head's AllToAll chunk as
                # soon as that head completes
                for unit in emit_att(3):
                    if unit is None:
                        continue
                    h = unit
                    nc.gpsimd.collective_compute(
                        "AllToAll",
                        mybir.AluOpType.bypass,
                        replica_groups=[list(range(N_CORES))],
                        ins=[a2a_in[h][:].opt()],
                        outs=[a2a_out[h][:].opt()],
                    )

            # ================= out projection ========================
            with (
                tc.tile_pool(name="p5ps", bufs=2, space="PSUM") as psC,
                tc.tile_pool(name="p5bc", bufs=2, space="PSUM") as psD,
                tc.tile_pool(name="p5sb", bufs=4) as t5,
                tc.tile_pool(name="an", bufs=1) as anp,
            ):
                R = ROWS_PER_CORE
                dsb_raw = [
                    t5.tile([2 * N_CORES, R], F32, tag=f"denraw{i}", name=f"denraw{i}")
                    for i in range(2)
                ]
                dsb_inv = [
                    t5.tile([2 * N_CORES, R], F32, tag=f"deninv{i}", name=f"deninv{i}")
                    for i in range(2)
                ]
                dsb = [
                    t5.tile([2 * N_CORES, R], F32R, tag=f"den{i}", name=f"den{i}")
                    for i in range(2)
                ]
                an_sb = anp.tile([128, 2 * N_CORES, R], BF16)

                def den_chunk(h):
                    hf, rs = h // 2, slice(8 * (h % 2), 8 * (h % 2) + 8)
                    nc.sync.dma_start(dsb_raw[hf][rs, :], a2a_out[h][:, 64, :])

                def an_half(half):
                    nc.vector.reciprocal_approx_fast(
                        out=dsb_inv[half][:, :], in_=dsb_raw[half][:, :]
                    )
                    nc.vector.tensor_copy(dsb[half][:, :], dsb_inv[half][:, :])
                    for g in range(N_CORES):
                        a_raw = t5.tile([128, R], F32, tag="araw")
                        nc.sync.dma_start(
                            a_raw[0:64, :], a2a_out[2 * half][g, 0:64, :]
                        )
                        nc.sync.dma_start(
                            a_raw[64:128, :], a2a_out[2 * half + 1][g, 0:64, :]
                        )
                        bc = psD.tile([128, R], F32, tag="bc")
                        nc.tensor.matmul(
                            bc[:],
                            sel_sb[:, 2 * g + half, :],
                            dsb[half][:, :],
                            start=True, stop=True,
                        )
                        nc.vector.tensor_mul(an_sb[:, 2 * g + half, :], a_raw[:], bc[:])

                den_chunk(0)
                den_chunk(1)
                an_half(0)
                den_chunk(2)
                den_chunk(3)
                an_half(1)

                NB_OUT = D_IN // 512  # 4
                for nb in range(NB_OUT):
                    osl = slice(512 * nb, 512 * nb + 512)
                    po = [
                        psC.tile([128, 512], F32, tag=f"o{q}", name=f"o{q}_{nb}")
                        for q in range(2)
                    ]
                    for half in range(2):
                        for g in range(N_CORES):
                            gh = 2 * g + half
                            first = half == 0 and g == 0
                            last = half == 1 and g == N_CORES - 1
                            for q in range(2):
                                nc.tensor.matmul(
                                    po[q][:],
                                    an_sb[:, gh, 128 * q : 128 * q + 128],
                                    wo_sb[:, gh, osl],
                                    start=first, stop=last,
                                )
                    for q in range(2):
                        osb = t5.tile([128, 512], F32, tag="osb")
                        nc.vector.tensor_copy(osb[:], po[q][:])
                        nc.sync.dma_start(out[128 * q : 128 * q + 128, osl], osb[:])

    nc.compile()
    return nc


_NC_CACHE = None


def _get_nc():
    global _NC_CACHE
    if _NC_CACHE is None:
        _NC_CACHE = _build()
    return _NC_CACHE


def _to_ktile_layout(w):
    """[D_IN, M] -> [128, KT_TILES, M] with row ko*128+p at [p, ko]."""
    m = w.shape[1]
    return np.ascontiguousarray(
        w.reshape(KT_TILES, 128, m).transpose(1, 0, 2)
    )


def _make_in_maps(x, cos, sin, wq, wk, wv, wo, q_norm_w, k_norm_w):
    x = np.asarray(x, dtype=np.float32)
    cos = np.asarray(cos, dtype=np.float32)
    sin = np.asarray(sin, dtype=np.float32)
    wq = np.asarray(wq, dtype=np.float32)
    wk = np.asarray(wk, dtype=np.float32)
    wv = np.asarray(wv, dtype=np.float32)
    wo = np.asarray(wo, dtype=np.float32)
    qw = np.asarray(q_norm_w, dtype=np.float32)
    kw = np.asarray(k_norm_w, dtype=np.float32)

    xT = np.ascontiguousarray(x[0].T).astype(BF16_NP)
    wo_b = _to_ktile_layout(wo).astype(BF16_NP)

    cosT = cos.T  # [64, SEQ]
    sinT = sin.T
    sgn = np.where(np.arange(64) < 32, -1.0, 1.0).astype(np.float32)
    wrot_q = qw[(np.arange(64) + 32) % 64]
    wrot_k = kw[(np.arange(64) + 32) % 64]
    cq1 = cosT * qw[:, None]
    sq1 = sinT * (sgn * wrot_q)[:, None]
    coswq = np.ascontiguousarray(np.vstack([cq1, cq1]))
    sinwq = np.ascontiguousarray(np.vstack([sq1, sq1]))
    coswk = np.ascontiguousarray(cosT * kw[:, None])
    sinwk = np.ascontiguousarray(sinT * (sgn * wrot_k)[:, None])

    ii, jj = np.meshgrid(np.arange(128), np.arange(128), indexing="ij")
    tri = np.where(ii <= jj, 0.0, NEG).astype(np.float32)
    onesblk = np.zeros((128, 128), np.float32)
    onesblk[0:64, 0:64] = 1.0
    onesblk[64:128, 64:128] = 1.0
    sel16 = np.zeros((2 * N_CORES, 2 * N_CORES, 128), np.float32)
    for g in range(N_CORES):
        for half in range(2):
            for m in range(128):
                sel16[8 * (m // 64) + g, 2 * g + half, m] = 1.0

    in_maps = []
    for c in range(N_CORES):
        wq_c = _to_ktile_layout(
            np.ascontiguousarray(wq[:, 256 * c : 256 * c + 256])
        ).astype(BF16_NP)
        wkv_c = _to_ktile_layout(
            np.ascontiguousarray(
                np.concatenate(
                    [wk[:, 64 * c : 64 * c + 64], wv[:, 64 * c : 64 * c + 64]],
                    axis=1,
                )
            )
        ).astype(BF16_NP)
        in_maps.append(
            {
                "xT": xT,
                "wq": wq_c,
                "wkv": wkv_c,
                "wo": wo_b,
                "coswq": coswq,
                "sinwq": sinwq,
                "coswk": coswk,
                "sinwk": sinwk,
                "tri": tri,
                "sel16": sel16,
                "onesblk": onesblk,
            }
        )
    return in_maps


def kernel(x, cos, sin, wq, wk, wv, wo, q_norm_w, k_norm_w):
    in_maps = _make_in_maps(x, cos, sin, wq, wk, wv, wo, q_norm_w, k_norm_w)
    nc = _get_nc()
    res = run_bass_kernel_spmd(nc, in_maps, core_ids=list(range(N_CORES)))
    rows = [res.results[c]["out"] for c in range(N_CORES)]
    full = np.concatenate(rows, axis=0)  # [SEQ, D_IN]
    return full.reshape(1, SEQ, D_IN).astype(np.float32)
